# revision 1
# baseline (speedup 1.0000x reference)
"""Trainium2 Bass kernel for nn_Decoder_1271310320240 (3-layer LSTM decoder).

Self-contained: builds a Bass/Tile program, shards the batch (1024 -> 8 x 128)
across 8 NeuronCores (data-parallel, weights replicated), runs SPMD via
bass_utils.run_bass_kernel_spmd, and reassembles the full output.

Per-core layout:
  - gates in PSUM as [B=128 partitions, 4d=1024] (two banks per layer),
    PyTorch gate column order [i, f, g, o].
  - matmuls out = lhsT.T @ rhs: lhsT = x^T / h^T slices [K=128, M=128] (fp16,
    stationary), rhs = W^T slices [K=128, N=512] (fp16, streaming). Bias via a
    K=1 matmul against a ones row.
  - ACT: sigmoid(i,f) 512-wide, tanh(g), sigmoid(o), tanh(c_new) per d-half.
  - DVE: fp16 elementwise, d-half-split tail; h fed back through 128x128 PE
    transposes (+ PSUM->SBUF copies) to rebuild h^T for the next matmuls.
  - h-phase (bias + W_hh) matmuls are emitted one layer-step early as PE
    filler under the elementwise tail.
  - All inputs host-packed into 3 DRAM tensors; a post-scheduling pass lowers
    multi-semaphore waits to single-wait NoOps (walrus encodes one wait per
    instruction).
"""

import sys
from contextlib import ExitStack

import numpy as np

sys.path.insert(0, "/opt/trn_rl_repo")

import concourse.bass as bass  # noqa: E402
import concourse.tile as tile  # noqa: E402
from concourse import mybir  # noqa: E402
from concourse.masks import make_identity  # noqa: E402

FP16 = mybir.dt.float16
F32 = mybir.dt.float32
AF = mybir.ActivationFunctionType

NL = 3
D = 256
BL = 128  # per-core batch
N_CORES = 8


TAU = 96


def build(tau=TAU, tail_split=2, tg_split=1, lower_waits=True, repeat=1):
    """Build the Bass module (single-core program, run SPMD on 8 cores).

    repeat>1 loops the tau-step recurrence repeat times with identical I/O
    (out[t % tau]) — used only for differential HW timing."""
    total = tau * repeat
    nc = bass.Bass("TRN2", target_bir_lowering=False, debug=False)

    # all inputs pre-packed on host into 3 tensors -> 3 DMAs, so no
    # instruction accumulates too many semaphore waits
    wt = nc.dram_tensor("wt", [128, 12 * 1024], FP16, kind="ExternalInput")
    bs = nc.dram_tensor("bs", [1, 3 * 1024], FP16, kind="ExternalInput")
    st = nc.dram_tensor("st", [128, 6 * 256], FP16, kind="ExternalInput")
    outp = nc.dram_tensor("out", [tau, 128, 256], F32, kind="ExternalOutput")

    out_ap = outp.ap()

    with ExitStack() as ctx:
        tc = ctx.enter_context(tile.TileContext(nc))
        consts = ctx.enter_context(tc.tile_pool(name="consts", bufs=1))
        state = ctx.enter_context(tc.tile_pool(name="state", bufs=2))
        acts = ctx.enter_context(tc.tile_pool(name="acts", bufs=2))
        outs = ctx.enter_context(tc.tile_pool(name="outs", bufs=3))
        psum = ctx.enter_context(tc.tile_pool(name="psum", bufs=1, space="PSUM"))
        psumT = ctx.enter_context(tc.tile_pool(name="psumT", bufs=2, space="PSUM"))

        # ---- constants (single DMA each) ----
        wtile = consts.tile([128, 12 * 1024], FP16, tag="wtile")
        # one DMA per layer so step-0 matmuls of layer 0 start as soon as
        # its weight block lands instead of waiting for the full 3 MB
        for l in range(NL):
            sl_w = slice(l * 4096, (l + 1) * 4096)
            nc.sync.dma_start(out=wtile[:, sl_w], in_=wt.ap()[:, sl_w])
        w = [[[wtile[:, ((l * 2 + m) * 2 + k) * 1024:((l * 2 + m) * 2 + k + 1) * 1024]
               for k in range(2)] for m in range(2)] for l in range(NL)]
        bstile = consts.tile([1, 3 * 1024], FP16, tag="bstile")
        nc.sync.dma_start(out=bstile, in_=bs.ap())
        bst = [bstile[:, l * 1024:(l + 1) * 1024] for l in range(NL)]
        sttile = consts.tile([128, 6 * 256], FP16, tag="sttile")
        nc.sync.dma_start(out=sttile, in_=st.ap())
        ones_t = consts.tile([1, 128], FP16, tag="ones")
        nc.gpsimd.memset(ones_t, 1.0)
        ident = consts.tile([128, 128], FP16, tag="ident")
        make_identity(nc, ident)

        # ---- state (step 0: DVE copies out of sttile, so step-0 elementwise
        # consumers only ever wait on one engine semaphore — the DVE
        # TensorTensor encoding supports a single sync wait) ----
        hT = [state.tile([128, 256], FP16, tag=f"hT{l}", name=f"hT{l}")
              for l in range(NL)]
        c = [state.tile([128, 256], FP16, tag=f"c{l}", name=f"c{l}")
             for l in range(NL)]
        for l in range(NL):
            nc.scalar.copy(hT[l], sttile[:, l * 256:(l + 1) * 256])
            nc.scalar.copy(c[l], sttile[:, 768 + l * 256:768 + (l + 1) * 256])

        pending_ps = [None] * NL  # psum tile with bias+hh accumulated
        ps_readers = [None] * NL  # ACT instructions that read the psum banks

        def h_phase(l):
            """bias + W_hh matmuls for layer l (next use of its psum bank)."""
            if ps_readers[l] is not None:
                # PE nop absorbing the WAR-on-ACT wait so the bias matmul
                # below needs only its single PE-drain wait (the HW
                # instruction encoding allows one semaphore wait).
                nop = nc.engines[mybir.EngineType.PE].nop(
                    nofuse=True, hint=f"war_absorb{l}")
                for rd in ps_readers[l]:
                    tile.add_dep_helper(nop.ins, rd.ins, sync=True,
                                        reason="absorb psum WAR")
            A = psum.tile([128, 512], F32, tag=f"psA{l}", name=f"psA{l}")
            Bk = psum.tile([128, 512], F32, tag=f"psB{l}", name=f"psB{l}")
            nc.tensor.matmul(A, ones_t, bst[l][:, 0:512], start=True, stop=False)
            nc.tensor.matmul(Bk, ones_t, bst[l][:, 512:1024], start=True, stop=False)
            nc.tensor.matmul(A, hT[l][:, 0:128], w[l][1][0][:, 0:512],
                             start=False, stop=False)
            nc.tensor.matmul(A, hT[l][:, 128:256], w[l][1][1][:, 0:512],
                             start=False, stop=False)
            nc.tensor.matmul(Bk, hT[l][:, 0:128], w[l][1][0][:, 512:1024],
                             start=False, stop=False)
            nc.tensor.matmul(Bk, hT[l][:, 128:256], w[l][1][1][:, 512:1024],
                             start=False, stop=False)
            pending_ps[l] = (A, Bk)

        # prologue: h-phases for step 0 layers 0 and 1
        h_phase(0)
        h_phase(1)

        for t in range(total):
            for l in range(NL):
                xT = hT[NL - 1] if l == 0 else hT[l - 1]
                A, Bk = pending_ps[l]
                # x-phase matmuls (critical path). i/f bank (A) first so
                # sigmoid(i,f) starts as early as possible; its k0 matmul
                # only needs the first transposed half of the previous h.
                nc.tensor.matmul(A, xT[:, 0:128], w[l][0][0][:, 0:512],
                                 start=False, stop=False)
                nc.tensor.matmul(A, xT[:, 128:256], w[l][0][1][:, 0:512],
                                 start=False, stop=True)
                nc.tensor.matmul(Bk, xT[:, 0:128], w[l][0][0][:, 512:1024],
                                 start=False, stop=False)
                nc.tensor.matmul(Bk, xT[:, 128:256], w[l][0][1][:, 512:1024],
                                 start=False, stop=True)

                # PE filler: h-phase of the layer-step 2 ahead
                nl_, nt_ = (l + 2) % NL, t + (l + 2) // NL
                if nt_ < total:
                    h_phase(nl_)

                # ACT: gate nonlinearities (i: 0:256, f: 256:512, g, o)
                sif = acts.tile([128, 512], FP16, tag="sif")
                i_sif = nc.scalar.activation(sif, A, AF.Sigmoid)
                tg = acts.tile([128, 256], FP16, tag="tg")
                i_tg = nc.scalar.activation(tg, Bk[:, 0:256], AF.Tanh)
                so = acts.tile([128, 256], FP16, tag="so")
                i_so = nc.scalar.activation(so, Bk[:, 256:512], AF.Sigmoid)
                ps_readers[l] = [i_sif, i_tg, i_so]

                # DVE: c_new = sig(f)*c + sig(i)*tanh(g)
                cn = state.tile([128, 256], FP16, tag=f"c{l}", name=f"cn{l}")
                h16 = acts.tile([128, 256], FP16, tag="h16")
                hTn = state.tile([128, 256], FP16, tag=f"hT{l}", name=f"hTn{l}")
                if tail_split == 2:
                    # fully d-half-split tail. DVE emission order matters
                    # (in-order queue): fc halves first (ready earliest),
                    # then ig/cn per half, then h/T/copy per half.
                    fcs = []
                    for hf in range(2):
                        sl_ = slice(hf * 128, (hf + 1) * 128)
                        sl_f = slice(256 + hf * 128, 256 + (hf + 1) * 128)
                        fc = acts.tile([128, 128], FP16, tag=f"fc{hf}",
                                       name=f"fc{hf}")
                        nc.vector.tensor_mul(fc, sif[:, sl_f], c[l][:, sl_])
                        fcs.append(fc)
                    for hf in range(2):
                        sl_ = slice(hf * 128, (hf + 1) * 128)
                        ig = acts.tile([128, 128], FP16, tag=f"ig{hf}",
                                       name=f"ig{hf}")
                        nc.vector.tensor_mul(ig, sif[:, sl_], tg[:, sl_])
                        nc.vector.tensor_add(cn[:, sl_], fcs[hf], ig)
                        tct = acts.tile([128, 128], FP16, tag=f"tc{hf}",
                                        name=f"tct{hf}")
                        nc.scalar.activation(tct, cn[:, sl_], AF.Tanh)
                        nc.vector.tensor_mul(h16[:, sl_], so[:, sl_], tct)
                        pst = psumT.tile([128, 128], FP16, tag=f"pst{hf}",
                                         name=f"pst{hf}", bufs=1)
                        nc.tensor.transpose(pst, h16[:, sl_], ident)
                        nc.vector.tensor_copy(hTn[:, sl_], pst)
                else:
                    fc = acts.tile([128, 256], FP16, tag="fc")
                    nc.vector.tensor_mul(fc, sif[:, 256:512], c[l])
                    ig = acts.tile([128, 256], FP16, tag="ig")
                    nc.vector.tensor_mul(ig, sif[:, 0:256], tg)
                    nc.vector.tensor_add(cn, fc, ig)
                    tct = acts.tile([128, 256], FP16, tag="tc")
                    nc.scalar.activation(tct, cn, AF.Tanh)
                    nc.vector.tensor_mul(h16, so, tct)
                    pst = psumT.tile([128, 256], FP16, tag="pst")
                    nc.tensor.transpose(pst[:, 0:128], h16[:, 0:128], ident)
                    nc.tensor.transpose(pst[:, 128:256], h16[:, 128:256], ident)
                    nc.vector.tensor_copy(hTn, pst)
                c[l] = cn
                hT[l] = hTn

                if l == NL - 1:
                    h32 = outs.tile([128, 256], F32, tag="h32")
                    nc.vector.tensor_copy(h32, h16)
                    nc.sync.dma_start(out=out_ap[t % tau], in_=h32)

    if lower_waits:
        _enforce_single_wait(nc)
    return nc


def _enforce_single_wait(nc):
    """Walrus only encodes ONE semaphore wait per compute instruction.

    The sequencer dispatches in order, so any wait on an earlier instruction
    of the same engine queue also gates every later instruction. Drop waits
    that are covered by earlier same-queue waits; the war_absorb nops emitted
    in the build guarantee coverage exists for the known 2-wait cases.
    """
    import concourse.mybir as mb
    fn = nc.m.functions[0]
    ctr = 0
    for blk in fn.blocks:
        cover = {}  # engine -> {sem_name: max value waited}
        out = []
        changed = False
        for ins in blk.instructions:
            si = ins.sync_info
            if si is not None and len(si.on_wait) > 1:
                eng = ins.engine
                cov = cover.setdefault(str(eng), {})
                kept = [w for w in si.on_wait
                        if not (w.wait_mode == "sem-ge-imm"
                                and cov.get(w.ant_name, -1) >= w.wait_value)]
                # extra waits become single-wait NoOps on the same queue
                for w in kept[:-1]:
                    ctr += 1
                    nop = mb.InstNoOp(
                        name=f"swx{ctr}", engine=eng,
                        sync_info=mb.SyncInfo(on_wait=[w], on_update=[]))
                    out.append(nop)
                    if w.wait_mode == "sem-ge-imm":
                        cov[w.ant_name] = max(cov.get(w.ant_name, -1),
                                              w.wait_value)
                ins.sync_info = mb.SyncInfo(on_wait=kept[-1:],
                                            on_update=list(si.on_update))
                changed = True
            si2 = ins.sync_info
            if si2 is not None and si2.on_wait:
                cov = cover.setdefault(str(getattr(ins, "engine", None)), {})
                for w in si2.on_wait:
                    if w.wait_mode == "sem-ge-imm":
                        cov[w.ant_name] = max(cov.get(w.ant_name, -1),
                                              w.wait_value)
            out.append(ins)
        if changed:
            blk.instructions = out


# ---------------- host-side pre/post-processing ----------------

def prep_inputs(hidden, cell, W_ih, W_hh, b_ih, b_hh):
    """Full inputs -> list of 8 per-core input maps (numpy)."""
    hidden = np.asarray(hidden, np.float32)
    cell = np.asarray(cell, np.float32)
    W_ih = np.asarray(W_ih, np.float32)
    W_hh = np.asarray(W_hh, np.float32)
    b_ih = np.asarray(b_ih, np.float32)
    b_hh = np.asarray(b_hh, np.float32)

    # weights packed [128, 12*1024]: col block (l,m,k) holds W_m[l][:, k*128+p].T
    wstk = np.stack([W_ih, W_hh], axis=1)            # [l, m, 4d, d]
    wtr = wstk.transpose(0, 1, 3, 2)                 # [l, m, d, 4d]
    wtr = wtr.reshape(NL, 2, 2, 128, 1024)           # [l, m, k, p, col]
    wt = wtr.transpose(3, 0, 1, 2, 4).reshape(128, 12 * 1024).astype(np.float16)
    bs = (b_ih + b_hh).reshape(1, 3 * 1024).astype(np.float16)

    in_maps = []
    for ci in range(N_CORES):
        sl = slice(ci * BL, (ci + 1) * BL)
        ht = hidden[:, sl, :].transpose(0, 2, 1)     # [l, d, b]
        ht = ht.reshape(NL, 2, 128, BL).transpose(2, 0, 1, 3).reshape(128, 768)
        cc = cell[:, sl, :].transpose(1, 0, 2).reshape(128, 768)  # [b, (l,d)]
        stt = np.concatenate([ht, cc], axis=1).astype(np.float16)
        in_maps.append({
            "wt": wt,
            "bs": bs,
            "st": np.ascontiguousarray(stt),
        })
    return in_maps


def assemble_output(results, tau=TAU):
    """list of per-core {"out": [tau,128,256] f32} -> [1024, tau, 256] f32."""
    full = np.empty((N_CORES * BL, tau, D), np.float32)
    for ci, r in enumerate(results):
        full[ci * BL:(ci + 1) * BL] = r["out"].transpose(1, 0, 2)
    return full


_NC_CACHE = {}


def _get_nc(tau):
    if tau not in _NC_CACHE:
        _NC_CACHE[tau] = build(tau)
    return _NC_CACHE[tau]


def kernel(hidden, cell, W_ih, W_hh, b_ih, b_hh, tau):
    from concourse.bass_utils import run_bass_kernel_spmd

    tau = int(np.asarray(tau))
    nc = _get_nc(tau)  # program is built (and cached) for the requested tau
    in_maps = prep_inputs(hidden, cell, W_ih, W_hh, b_ih, b_hh)
    res = run_bass_kernel_spmd(nc, in_maps, core_ids=list(range(N_CORES)))
    return assemble_output(res.results, tau)



# revision 2
# speedup vs baseline: 1.0619x; 1.0619x over previous
"""Trainium2 Bass kernel v3 for nn_Decoder_1271310320240 (3-layer LSTM decoder).

Transposed-state design, sigmoid i/f gates + tanh-trick o gate:
  - States stored transposed: h~T = [128 part = d-half rows, 2*128 = (d-half,
    batch)] with h~ = 2h; c stored unscaled, same layout.
  - Gates computed as gates^T = W'^T x~^T directly in PSUM [gate rows, batch].
    Per layer two banks: bank0 = [i0 f0 i1 f1], bank1 = [g0 g1 o0 o1]
    (suffix = d-half). Weights stationary; no PE transposes anywhere.
  - ACT ops per cell: Sigmoid(bank0) [512], Tanh(g01) [256], Tanh(o01) [256],
    Tanh(cn_j) [128] x2. The o-gate uses tanh(o/2) so h~ = (to+1)*tc = 2h
    (the /2 folded into host-packed W'); x~=2h comp. folds W/2 everywhere.
  - DVE: fc=sf*c, ig=si*tg, cn=fc+ig (plain tensor_tensor, 2x fp16 mode),
    h~_j = (to_j+1)*tc_j via scalar_tensor_tensor.
  - Bias via one K=4 indicator matmul per bank (start=True), issued with the
    off-critical-path h-phase (emitted 2 cells ahead).
  - Output: Pool engine scales h~T by 0.5 into f32, DMA out transposed; host
    reassembles [B, tau, D].
"""

import sys
from contextlib import ExitStack

import numpy as np

sys.path.insert(0, "/opt/trn_rl_repo")

import concourse.bass as bass  # noqa: E402
import concourse.tile as tile  # noqa: E402
from concourse import mybir  # noqa: E402

FP16 = mybir.dt.float16
F32 = mybir.dt.float32
AF = mybir.ActivationFunctionType
ALU = mybir.AluOpType

NL = 3
D = 256
BL = 128  # per-core batch
N_CORES = 8
TAU = 96

# tile index -> (gate, d-half): bank0 = [i0 f0 i1 f1], bank1 = [g0 g1 o0 o1]
TILES = [(0, 0), (1, 0), (0, 1), (1, 1), (2, 0), (2, 1), (3, 0), (3, 1)]


def build(tau=TAU, lower_waits=True, repeat=1):
    """Build the Bass module (single-core program, run SPMD on 8 cores).

    repeat>1 loops the recurrence with identical I/O (out[t % tau]) — used
    only for differential HW timing."""
    total = tau * repeat
    nc = bass.Bass("TRN2", target_bir_lowering=False, debug=False)

    # weights: per layer 4096 cols: [x-phase 16 tiles | h-phase 16 tiles],
    # tile (j, g, k) at ((j*4+g)*2+k)*128 within the phase block
    wt = nc.dram_tensor("wt", [128, NL * 4096], FP16, kind="ExternalInput")
    bt = nc.dram_tensor("bt", [4, NL * 256], FP16, kind="ExternalInput")
    ind = nc.dram_tensor("ind", [4, 512], FP16, kind="ExternalInput")
    st = nc.dram_tensor("st", [128, 6 * 256], FP16, kind="ExternalInput")
    outp = nc.dram_tensor("out", [tau, 128, 256], F32, kind="ExternalOutput")
    out_ap = outp.ap()

    with ExitStack() as ctx:
        tc = ctx.enter_context(tile.TileContext(nc))
        consts = ctx.enter_context(tc.tile_pool(name="consts", bufs=1))
        state = ctx.enter_context(tc.tile_pool(name="state", bufs=2))
        acts = ctx.enter_context(tc.tile_pool(name="acts", bufs=2))
        outs = ctx.enter_context(tc.tile_pool(name="outs", bufs=3))
        psum = ctx.enter_context(tc.tile_pool(name="psum", bufs=1, space="PSUM"))

        # ---- constants ----
        wtile = consts.tile([128, NL * 4096], FP16, tag="wtile")
        for l in range(NL):
            sl = slice(l * 4096, (l + 1) * 4096)
            nc.sync.dma_start(out=wtile[:, sl], in_=wt.ap()[:, sl])
        btile = consts.tile([4, NL * 256], FP16, tag="btile")
        nc.sync.dma_start(out=btile, in_=bt.ap())
        indt = consts.tile([4, 512], FP16, tag="indt")
        nc.sync.dma_start(out=indt, in_=ind.ap())
        sttile = consts.tile([128, 6 * 256], FP16, tag="sttile")
        nc.sync.dma_start(out=sttile, in_=st.ap())

        def wap(l, m, idx, k):
            base = l * 4096 + m * 2048 + (idx * 2 + k) * 128
            return wtile[:, base:base + 128]

        def bap(l, bank):
            base = l * 256 + bank * 128
            return btile[:, base:base + 128]

        # ---- state (step 0) ----
        hT = [state.tile([128, 256], FP16, tag=f"hT{l}", name=f"hT{l}")
              for l in range(NL)]
        c = [state.tile([128, 256], FP16, tag=f"c{l}", name=f"c{l}")
             for l in range(NL)]
        for l in range(NL):
            nc.scalar.copy(hT[l], sttile[:, l * 256:(l + 1) * 256])
            nc.scalar.copy(c[l], sttile[:, 768 + l * 256:768 + (l + 1) * 256])

        pending = {}     # cell n -> (ps_j0, ps_j1)
        act1_insts = {}  # cell n -> [ACT1_0, ACT1_1] (psum readers, for WAR)

        def h_phase(n):
            """bias + W_hh matmuls for cell n (uses h~ of cell n-3 = same
            layer previous step, or initial state)."""
            t, l = divmod(n, NL)
            if t >= total:
                return
            # absorb the WAR wait on the previous ACT1 readers of this
            # layer's psum banks into a PE nop, so the bias matmul below
            # carries a single encodable wait.
            rd = act1_insts.pop(n - NL, None)
            if rd is not None:
                nop = nc.engines[mybir.EngineType.PE].nop(
                    nofuse=True, hint=f"war_absorb{n}")
                for r in rd:
                    tile.add_dep_helper(nop.ins, r.ins, sync=True,
                                        reason="absorb psum WAR")
            ps = []
            for bank in range(2):
                p = psum.tile([128, 512], F32, tag=f"ps{l}{bank}",
                              name=f"ps{l}{bank}_{t}")
                nc.tensor.matmul(p, bap(l, bank), indt, start=True, stop=False)
                ps.append(p)
            for k in range(2):
                rhs = hT[l][:, k * 128:(k + 1) * 128]
                for idx in range(8):
                    nc.tensor.matmul(
                        ps[idx // 4][:, (idx % 4) * 128:(idx % 4 + 1) * 128],
                        wap(l, 1, idx, k), rhs,
                        start=False, stop=False)
            pending[n] = ps

        # prologue: h-phases for cells 0 and 1
        h_phase(0)
        h_phase(1)

        for n in range(NL * total):
            t, l = divmod(n, NL)
            xT = hT[l - 1] if l > 0 else hT[NL - 1]
            ps = pending.pop(n)

            # x-phase matmuls. The previous cell computes h~ half1 BEFORE
            # half0, so k=1 contractions can start first. i/f tiles first so
            # Sigmoid(bank0) starts as early as possible. Emission order:
            # [k1-if, k0-if, k1-go, k0-go]; stop on the last write per bank.
            for idx_group in ((2, 3, 0, 1), (4, 5, 6, 7)):
                for k in (1, 0):
                    rhs = xT[:, k * 128:(k + 1) * 128]
                    for idx in idx_group:
                        nc.tensor.matmul(
                            ps[idx // 4][:, (idx % 4) * 128:(idx % 4 + 1) * 128],
                            wap(l, 0, idx, k), rhs,
                            start=False, stop=(k == 0 and idx == idx_group[-1]))

            # PE filler: h-phase of the cell 2 ahead (same dependency as
            # this cell's x-phase: h~ of cell n-1)
            h_phase(n + 2)

            # ACT: sigmoid(bank0 = [i0 f0 i1 f1]) split into two 256-wide ops
            # (HW: 2x186 << 1x707), half 1 first; then tanh(g01), tanh(o01)
            sif = acts.tile([128, 512], FP16, tag="sif", name=f"sif_{n}")
            a_sif1 = nc.scalar.activation(sif[:, 256:512], ps[0][:, 256:512],
                                          AF.Sigmoid)
            a_sif0 = nc.scalar.activation(sif[:, 0:256], ps[0][:, 0:256],
                                          AF.Sigmoid)
            tgg = acts.tile([128, 256], FP16, tag="tgg", name=f"tgg_{n}")
            a_tg = nc.scalar.activation(tgg, ps[1][:, 0:256], AF.Tanh)
            tot = acts.tile([128, 256], FP16, tag="tot", name=f"tot_{n}")
            a_to = nc.scalar.activation(tot, ps[1][:, 256:512], AF.Tanh)
            act1_insts[n] = [a_sif1, a_sif0, a_tg, a_to]

            cn = state.tile([128, 256], FP16, tag=f"c{l}", name=f"cn{l}_{t}")
            hTn = state.tile([128, 256], FP16, tag=f"hT{l}", name=f"hTn{l}_{t}")
            tct = [acts.tile([128, 128], FP16, tag=f"tc{j}", name=f"tc{j}_{n}")
                   for j in range(2)]
            # DVE: fc_j = sf_j*c_j early for both halves (half 1 first, its
            # sigmoid lands first), then ig/cn with half 1 FIRST (its h~
            # unblocks the next cell's k1 matmuls).
            fc = [None, None]
            for j in (1, 0):
                sj = slice(j * 128, (j + 1) * 128)
                fcj = acts.tile([128, 128], FP16, tag=f"fc{j}", name=f"fc{j}_{n}")
                nc.vector.tensor_mul(fcj, sif[:, j * 256 + 128:(j + 1) * 256],
                                     c[l][:, sj])
                fc[j] = fcj
            for j in (1, 0):
                sj = slice(j * 128, (j + 1) * 128)
                igj = acts.tile([128, 128], FP16, tag=f"ig{j}", name=f"ig{j}_{n}")
                nc.vector.tensor_mul(igj, sif[:, j * 256:j * 256 + 128],
                                     tgg[:, sj])
                nc.vector.tensor_add(cn[:, sj], fc[j], igj)
                # ACT: tc_j = tanh(cn_j)
                nc.scalar.activation(tct[j], cn[:, sj], AF.Tanh)
            # DVE: top = to+1 (4x-mode TS), then h~_j = top_j*tc_j (2x TT)
            top = acts.tile([128, 256], FP16, tag="top", name=f"top_{n}")
            nc.vector.tensor_scalar_add(top, tot, 1.0)
            for j in (1, 0):
                sj = slice(j * 128, (j + 1) * 128)
                nc.vector.tensor_mul(hTn[:, sj], top[:, sj], tct[j])
            c[l] = cn
            hT[l] = hTn

            if l == NL - 1:
                h32 = outs.tile([128, 256], F32, tag="h32", name=f"h32_{t}")
                nc.gpsimd.tensor_scalar_mul(h32, hTn, 0.5)
                nc.sync.dma_start(out=out_ap[t % tau], in_=h32)

    if lower_waits:
        _enforce_single_wait(nc)
    return nc


def _enforce_single_wait(nc):
    """Walrus only encodes ONE semaphore wait per compute instruction.

    Drop waits covered by earlier same-queue waits; lower remaining extras
    to single-wait NoOps on the same queue."""
    import concourse.mybir as mb
    fn = nc.m.functions[0]
    ctr = 0
    for blk in fn.blocks:
        cover = {}
        out = []
        changed = False
        for ins in blk.instructions:
            si = ins.sync_info
            if si is not None and len(si.on_wait) > 1:
                eng = ins.engine
                cov = cover.setdefault(str(eng), {})
                kept = [w for w in si.on_wait
                        if not (w.wait_mode == "sem-ge-imm"
                                and cov.get(w.ant_name, -1) >= w.wait_value)]
                if not kept:
                    kept = list(si.on_wait)[-1:]
                for w in kept[:-1]:
                    ctr += 1
                    nop = mb.InstNoOp(
                        name=f"swx{ctr}", engine=eng,
                        sync_info=mb.SyncInfo(on_wait=[w], on_update=[]))
                    out.append(nop)
                    if w.wait_mode == "sem-ge-imm":
                        cov[w.ant_name] = max(cov.get(w.ant_name, -1),
                                              w.wait_value)
                ins.sync_info = mb.SyncInfo(on_wait=kept[-1:],
                                            on_update=list(si.on_update))
                changed = True
            si2 = ins.sync_info
            if si2 is not None and si2.on_wait:
                cov = cover.setdefault(str(getattr(ins, "engine", None)), {})
                for w in si2.on_wait:
                    if w.wait_mode == "sem-ge-imm":
                        cov[w.ant_name] = max(cov.get(w.ant_name, -1),
                                              w.wait_value)
            out.append(ins)
        if changed:
            blk.instructions = out


# ---------------- host-side pre/post-processing ----------------

def prep_inputs(hidden, cell, W_ih, W_hh, b_ih, b_hh):
    """Full inputs -> list of 8 per-core input maps (numpy)."""
    hidden = np.asarray(hidden, np.float32)
    cell = np.asarray(cell, np.float32)
    W_ih = np.asarray(W_ih, np.float32)
    W_hh = np.asarray(W_hh, np.float32)
    b_ih = np.asarray(b_ih, np.float32)
    b_hh = np.asarray(b_hh, np.float32)

    # gate scaling: all gates get the x~=2h compensation (W/2); o-gate gets
    # an extra /2 (tanh(o/2) trick => h~ = (tanh+1)*tc = 2h). Biases: o/2.
    w_scale = np.array([0.5, 0.5, 0.5, 0.25], np.float32)
    b_scale = np.array([1.0, 1.0, 1.0, 0.5], np.float32)
    _T = TILES

    # wt: [128 K rows, NL*4096]; tile (l, m, idx, k): lhsT[p, q] =
    #   W'_m[l, g*256 + j*128 + q, k*128 + p], (g, j) = TILES[idx]
    wt = np.empty((128, NL * 4096), np.float32)
    for l in range(NL):
        for m, W in enumerate((W_ih, W_hh)):
            Wl = W[l].reshape(4, 2, 128, 2, 128)  # [g, j, q, k, p]
            Wl = Wl * w_scale[:, None, None, None, None]
            for idx, (g, j) in enumerate(_T):
                for k in range(2):
                    base = l * 4096 + m * 2048 + (idx * 2 + k) * 128
                    wt[:, base:base + 128] = Wl[g, j, :, k, :].T
    wt = wt.astype(np.float16)

    b = (b_ih + b_hh).reshape(NL, 4, 2, 128) * b_scale[None, :, None, None]
    # bt: [4, NL*256]; (l, bank) block cols l*256+bank*128:
    #   bt[r, ...+q] = b'[l, gate, half, q] for tile idx = bank*4+r
    bt = np.empty((4, NL * 256), np.float32)
    for l in range(NL):
        for idx, (g, j) in enumerate(_T):
            bank, r = divmod(idx, 4)
            bt[r, l * 256 + bank * 128:l * 256 + (bank + 1) * 128] = b[l, g, j]
    bt = bt.astype(np.float16)

    ind = np.zeros((4, 512), np.float16)
    for g in range(4):
        ind[g, g * 128:(g + 1) * 128] = 1.0

    in_maps = []
    for ci in range(N_CORES):
        sl = slice(ci * BL, (ci + 1) * BL)
        # h~T[p, k*128 + b] = 2*h[l, b, k*128+p]
        ht = 2.0 * hidden[:, sl, :]                # [l, b, d]
        ht = ht.reshape(NL, BL, 2, 128).transpose(0, 3, 2, 1)  # [l, p, k, b]
        ht = ht.reshape(NL, 128, 256).transpose(1, 0, 2).reshape(128, NL * 256)
        cc = cell[:, sl, :]
        cc = cc.reshape(NL, BL, 2, 128).transpose(0, 3, 2, 1)
        cc = cc.reshape(NL, 128, 256).transpose(1, 0, 2).reshape(128, NL * 256)
        stt = np.concatenate([ht, cc], axis=1).astype(np.float16)
        in_maps.append({
            "wt": wt,
            "bt": bt,
            "ind": ind,
            "st": np.ascontiguousarray(stt),
        })
    return in_maps


def assemble_output(results, tau=TAU):
    """per-core {"out": [tau,128,256] f32} -> [1024, tau, 256] f32.

    out[t, p, k*128+b] = h[b, k*128+p]."""
    full = np.empty((N_CORES * BL, tau, D), np.float32)
    for ci, r in enumerate(results):
        o = r["out"].reshape(tau, 128, 2, BL)       # [t, p, k, b]
        full[ci * BL:(ci + 1) * BL] = o.transpose(3, 0, 2, 1).reshape(BL, tau, D)
    return full


_NC_CACHE = {}


def _get_nc(tau):
    if tau not in _NC_CACHE:
        _NC_CACHE[tau] = build(tau)
    return _NC_CACHE[tau]


def kernel(hidden, cell, W_ih, W_hh, b_ih, b_hh, tau):
    from concourse.bass_utils import run_bass_kernel_spmd

    tau = int(np.asarray(tau))
    nc = _get_nc(tau)
    in_maps = prep_inputs(hidden, cell, W_ih, W_hh, b_ih, b_hh)
    res = run_bass_kernel_spmd(nc, in_maps, core_ids=list(range(N_CORES)))
    return assemble_output(res.results, tau)
